# revision 4
# baseline (speedup 1.0000x reference)
"""Trainium2 Bass kernel for nn_DirectionalConvLayer.

Model (from the reference): per sample, a forward then backward scan over h;
each step = 3x3 conv on a single row (only the middle kernel row W[:,:,1,:]
contributes), + bias b, InstanceNorm over the row, ELU, + residual row. The
conv bias b cancels exactly under InstanceNorm and is never sent the device.

Sharding: data-parallel over batch n=8 -> one sample per NeuronCore, no
collectives. Each core runs the sequential 2*(h-1)-step scan on its sample.

Step math, with z = conv(prev row), m/v = row stats of z, rs = rsqrt(v+eps),
u = (z-m)*rs (standardized, so exp never overflows):
    f_dst = elu(u) + resid = [min(exp(u),1) - 1] + [relu(u)] + resid
          = em - 1 + ru*rs + resid
      em = exp(rs * min(z-m, 0)),  ru = max(z-m, 0)

Key structure (v2): the conv is linear, so the next step's conv input is kept
as a 128-partition "combo" slot holding em on partitions 0:64 and
q = ru*rs + (resid - 1) on partitions 64:128; one K=128 matmul per tap then
computes conv(em) + conv(q) together. The combo sums to f-1 (forward; pads
set to -1) or g+1 (backward; pads +1), making the conv result differ from
the true z only by a per-channel constant, which InstanceNorm's mean
subtraction absorbs exactly. This lets the three matmuls of step i+1 start
as soon as em/q land, while the fp32 stack write (clean f resp. g, used for
the backward residual and the final output) happens off the critical path.

All ACT functions used (Ln, Exp, Copy) live in one activation-table set
(natural_log_exp_and_others); the default chooser would reload tables twice
per step (~2.6us/step), so _Bacc pins the choice.

Per-step engine budget: PE 3 matmuls; DVE bn_stats/aggr + rd/ru (from an
SBUF copy of z, 2x perf mode) + q + stack write; ACT z-copy, Ln, Exp(rs),
Exp(em) -> combo and a duplicate em2 at partitions 64:128 so the stack write
reads same-base inputs (HW requires equal base partitions for multi-tensor
SBUF inputs; output base is free).
"""

from contextlib import ExitStack

import numpy as np

import concourse.bacc as bacc
import concourse.bass as bass
import concourse.mybir as mybir
import concourse.tile as tile
from concourse.bass_utils import run_bass_kernel_spmd

F32 = mybir.dt.float32
F16 = mybir.dt.float16
AF = mybir.ActivationFunctionType
OP = mybir.AluOpType

# fp16 combo slots: conv inputs (em/q) and weights in fp16 -> fast weight
# loads + cheaper matmuls; values are bounded (em in [0,1], |q| < ~20) so
# fp16 range is safe and its 10-bit mantissa keeps the end-to-end error
# ~3e-4. The fp32 stack (residual chain + output) is fed by fp32 duplicates
# (em2/q32), so only each step's elu-term is quantized.
COMBO_BF16 = True

EPS = 1e-5
C = 64          # channels
WDIM = 256      # row width
H_FULL = 256    # rows
SLOTW = WDIM + 2  # padded row slot width
XAHEAD = 4      # resid DMA prefetch distance
OUTB = 4        # out rows per store DMA
NCOMBO = 3      # combo ring depth
SBUFS = 3       # stats pool depth
USE_ZC = True   # stage z in SBUF via ACT copy (False: DVE reads PSUM direct)
ZDUP = False    # write z twice (2nd PSUM bank) to unserialize bank readers
ZC_MID = False  # emit the z-copy between Ln and Exp on ACT
ZC_FIRST = True  # emit the z-copy before bn_stats (parallel on ACT)
EBUFS = 3       # elementwise pool depth
ZBUFS = 2       # z PSUM pool depth


class _Bacc(bacc.Bacc):
    """Bacc whose activation-table chooser is forced to the single set
    containing Ln and Exp (natural_log_exp_and_others); the default
    first-match rule alternates natural_log / exp_and_others, reloading
    ACT tables twice per scan step."""

    def insert_act_table_loads(self):
        import bass_rust as _bass_rust
        from concourse.hw_specs import get_activation_tables

        has_activation = any(
            isinstance(i, mybir.InstActivation)
            for b in self.main_func.blocks
            for i in b.instructions
        )
        if not has_activation:
            return
        want = {AF.Ln, AF.Exp, AF.Copy}
        tables = [
            (name, funcs if name == "natural_log_exp_and_others"
             else funcs - want)
            for name, funcs in get_activation_tables(self.m.arch).items()
        ]
        _bass_rust.insert_act_table_loads(self, tables)


def _build(h=H_FULL, combo_bf16=COMBO_BF16):
    cdt = F16 if combo_bf16 else F32
    half_rows = h // 2

    def hb(row):  # partition base of the stack half that owns `row`
        return 0 if row < half_rows else 64

    def soff(row):  # column offset of `row`'s slot within its half
        return (row % half_rows) * SLOTW

    nc = _Bacc("TRN2", target_bir_lowering=False, debug=False, num_devices=8)
    # xm1 = x - 1 (host-side); the -1 belongs inside q = ru*rs + resid - 1
    xm1 = nc.dram_tensor("xm1", [C, h, WDIM], F32, kind="ExternalInput").ap()
    # wb[half, ci, k, co] = W[co, ci, 1, k]; duplicated across both halves ->
    # combo matmuls contract em (parts 0:64) and q (parts 64:128) in one shot
    wb = nc.dram_tensor("wb", [2, C, 3, C], F32, kind="ExternalInput").ap()
    out = nc.dram_tensor("out", [C, h, WDIM], F32, kind="ExternalOutput").ap()

    with tile.TileContext(nc) as tc, ExitStack() as ctx:
        singles = ctx.enter_context(tc.tile_pool(name="singles", bufs=1))
        spool = ctx.enter_context(tc.tile_pool(name="stats", bufs=SBUFS))
        epool = ctx.enter_context(tc.tile_pool(name="elems", bufs=EBUFS))
        xpool = ctx.enter_context(tc.tile_pool(name="xrows", bufs=XAHEAD + 2))
        zpool = ctx.enter_context(tc.tile_pool(name="zpsum", bufs=ZBUFS, space="PSUM"))

        stack = singles.tile([128, half_rows * SLOTW], F32)
        w_both = singles.tile([128, 3 * C], F32)
        w_stage = singles.tile([128, 3 * C], F32)
        eps_t = singles.tile([128, 1], F32)
        nc.vector.memset(eps_t, EPS)

        # stage weights through a DVE copy so matmul weight deps are DVE ticks
        nc.sync.dma_start(out=w_stage, in_=wb)
        nc.vector.tensor_copy(w_both, w_stage)
        if combo_bf16:
            # fp16 weights for the steady (combo) taps; seeds stay fp32
            w16 = singles.tile([128, 3 * C], F16)
            nc.vector.tensor_copy(w16, w_stage)
        else:
            w16 = w_both

        # combo ring: persistent tiles. The em+q SUM over the two halves must
        # equal (f-1) resp. (g+1) per column including pads, so the phase pad
        # value lives on the q half only; em-half pads stay 0.
        combos = [singles.tile([128, SLOTW], cdt, name=f"combo{j}")
                  for j in range(NCOMBO)]
        for cb in combos:
            nc.vector.memset(cb[0:64, 0:1], 0.0)
            nc.vector.memset(cb[0:64, SLOTW - 1:SLOTW], 0.0)

        def set_pads(val):
            for cb in combos:
                nc.vector.memset(cb[64:128, 0:1], val)
                nc.vector.memset(cb[64:128, SLOTW - 1:SLOTW], val)

        # forward: q = ru*rs + (x-1) already carries elu's -1, so the combo
        # sums to exactly f -> pads stay 0 and the stack write adds 0
        set_pads(0.0)

        # stack pad columns are zero (seed rows are conv'd directly)
        stack3 = stack.rearrange("p (s c) -> p s c", c=SLOTW)
        nc.vector.memset(stack3[:, :, 0:1], 0.0)
        nc.vector.memset(stack3[:, :, SLOTW - 1:SLOTW], 0.0)

        # f[0] = x[0] = xm1[0] + 1, staged then fixed up on DVE
        x0 = xpool.tile([128, WDIM], F32, tag="xr")
        nc.sync.dma_start(out=x0[0:64, :], in_=xm1[:, 0, :])
        nc.vector.tensor_scalar_add(stack[0:64, 1:WDIM + 1], x0[0:64, :], 1.0)

        resid_tiles = {}

        def fetch_xrow(row):
            xr = xpool.tile([128, WDIM], F32, tag="xr", name=f"xr{row}")
            nc.sync.dma_start(out=xr[0:64, :], in_=xm1[:, row, :])
            resid_tiles[row] = xr

        def fetch_resid_bwd(row):
            # backward resid is stack row `row` (clean f); rows in the upper
            # half live at partitions 64:128 but the q-stt needs base-0
            # inputs -> stage through an SBUF->SBUF DMA. Lower half reads
            # the stack directly.
            if hb(row) == 0:
                resid_tiles[row] = None  # direct
                return
            xr = xpool.tile([128, WDIM], F32, tag="xr", name=f"br{row}")
            nc.sync.dma_start(
                out=xr[0:64, :],
                in_=stack[64:128, soff(row) + 1:soff(row) + 1 + WDIM],
            )
            resid_tiles[row] = xr

        def resid_ap(row):
            t = resid_tiles.pop(row)
            if t is None:
                return stack[0:64, soff(row) + 1:soff(row) + 1 + WDIM]
            return t[0:64, :]

        def step(dst, src_combo, src_stack_row, stack_scalar):
            """One scan step. Conv input: either a combo ring slot (K=128,
            em+q) or a stack row (seed steps, K=64). Writes em/q into
            combo[dst % NCOMBO] and the clean row into stack slot dst."""
            hd = hb(dst)
            do = soff(dst)
            zt = zpool.tile([128, WDIM], F32, tag="z", name=f"z{dst}")
            z = zt[0:64, :]
            ztargets = [z]
            if ZDUP:
                zt2 = zpool.tile([128, WDIM], F32, tag="zb", name=f"zb{dst}")
                ztargets.append(zt2[0:64, :])
            for zdst in ztargets:
                if src_combo is not None:
                    for k in range(3):
                        nc.tensor.matmul(
                            zdst,
                            lhsT=w16[:, k * C:(k + 1) * C],
                            rhs=src_combo[:, k:k + WDIM],
                            start=(k == 0),
                            stop=(k == 2),
                        )
                else:
                    hs = hb(src_stack_row)
                    so = soff(src_stack_row)
                    for k in range(3):
                        nc.tensor.matmul(
                            zdst,
                            lhsT=w_both[hs:hs + 64, k * C:(k + 1) * C],
                            rhs=stack[hs:hs + 64, so + k:so + k + WDIM],
                            start=(k == 0),
                            stop=(k == 2),
                        )
            z2 = ztargets[-1]

            if USE_ZC and ZC_FIRST:
                # emit the z copy BEFORE bn_stats: walrus then encodes its
                # z-dependency directly on the PE sem, so it runs on ACT in
                # parallel with bn_stats instead of serializing behind it
                zsrc_t = epool.tile([128, WDIM], F32, tag="zc", name=f"zc{dst}")
                nc.scalar.activation(zsrc_t[0:64, :], z, AF.Copy)
                zsrc = zsrc_t[0:64, :]

            st6 = spool.tile([128, 6], F32, tag="st6", name=f"st{dst}")
            nc.vector.bn_stats(st6[0:64, :], z)
            mv = spool.tile([128, 2], F32, tag="mv", name=f"mv{dst}")
            nc.vector.bn_aggr(mv[0:64, :], st6[0:64, :])
            mean = mv[0:64, 0:1]
            var = mv[0:64, 1:2]
            lv = spool.tile([128, 1], F32, tag="lv", name=f"lv{dst}")
            nc.scalar.activation(lv[0:64, :], var, AF.Ln, bias=eps_t[0:64, :])
            rs = spool.tile([128, 1], F32, tag="rs", name=f"rs{dst}")
            if USE_ZC and ZC_MID and not ZC_FIRST:
                zsrc_t = epool.tile([128, WDIM], F32, tag="zc", name=f"zc{dst}")
                nc.scalar.activation(zsrc_t[0:64, :], z2, AF.Copy)
                zsrc = zsrc_t[0:64, :]
            nc.scalar.activation(rs[0:64, :], lv[0:64, :], AF.Exp, scale=-0.5)
            if USE_ZC and not ZC_MID and not ZC_FIRST:
                # z copy to SBUF: unlocks DVE 2x perf mode for rd/ru, but
                # serializes them behind the ACT queue
                zsrc_t = epool.tile([128, WDIM], F32, tag="zc", name=f"zc{dst}")
                nc.scalar.activation(zsrc_t[0:64, :], z, AF.Copy)
                zsrc = zsrc_t[0:64, :]
            if not USE_ZC:
                zsrc = z2
            rd = epool.tile([128, WDIM], F32, tag="rd", name=f"rd{dst}")
            nc.vector.tensor_scalar(
                rd[0:64, :], zsrc, mean, 0.0, OP.subtract, OP.min
            )
            ru = epool.tile([128, WDIM], F32, tag="ru", name=f"ru{dst}")
            nc.vector.tensor_scalar(
                ru[0:64, :], zsrc, mean, 0.0, OP.subtract, OP.max
            )
            cb = combos[dst % NCOMBO]
            # em = exp(rs*min(z-m,0)) = min(exp(u),1), into combo em half
            nc.scalar.activation(
                cb[0:64, 1:WDIM + 1], rd[0:64, :], AF.Exp, scale=rs[0:64, :]
            )
            # q = ru*rs + resid(-1) into combo q half
            resid = resid_ap(dst)
            nc.vector.scalar_tensor_tensor(
                cb[64:128, 1:WDIM + 1], ru[0:64, :], rs[0:64, :],
                resid, OP.mult, OP.add,
            )
            # em duplicate at partitions 64:128 so the stack write reads
            # same-base inputs
            em2 = epool.tile([128, WDIM], F32, tag="em2", name=f"em2{dst}")
            nc.scalar.activation(
                em2[64:128, :], rd[0:64, :], AF.Exp, scale=rs[0:64, :]
            )
            if combo_bf16:
                # fp32 duplicate of q for the stack write (combo q is f16)
                qsrc = epool.tile([128, WDIM], F32, tag="q32", name=f"q32{dst}")
                nc.vector.scalar_tensor_tensor(
                    qsrc[64:128, :], ru[0:64, :], rs[0:64, :],
                    resid, OP.mult, OP.add,
                )
                qsrc = qsrc[64:128, :]
            else:
                qsrc = cb[64:128, 1:WDIM + 1]
            # stack <- em + q + stack_scalar (clean f fwd / clean g bwd), on
            # GPSIMD (plain TT / scalar-imm forms only; stt is rejected on
            # Pool by walrus) - SBUF-only and off the critical path
            fdst = stack[hd:hd + 64, do + 1:do + 1 + WDIM]
            if stack_scalar == 0.0:
                nc.gpsimd.tensor_tensor(fdst, em2[64:128, :], qsrc, OP.add)
            else:
                tmp = epool.tile([128, WDIM], F32, tag="fs", name=f"fs{dst}")
                nc.gpsimd.tensor_tensor(
                    tmp[64:128, :], em2[64:128, :], qsrc, OP.add
                )
                nc.gpsimd.tensor_scalar_add(fdst, tmp[64:128, :], stack_scalar)
            return cb

        # ---- forward scan: f[i] = elu-step(f[i-1]) + x[i] ----
        for rr in range(1, min(1 + XAHEAD, h)):
            fetch_xrow(rr)
        prev_combo = None
        for i in range(1, h):
            if i + XAHEAD <= h - 1:
                fetch_xrow(i + XAHEAD)
            # forward: resid = xm1 -> combo sums to f -> stack scalar 0
            prev_combo = step(
                i,
                prev_combo if i > 1 else None,
                i - 1,
                0.0,
            )

        # ---- backward scan: g[p] = elu-step(g[p+1]) + f[p] ----
        set_pads(1.0)  # backward: combo = g + 1

        def store_rows(p0):
            hd = hb(p0)
            src = stack[hd:hd + 64, :].rearrange("p (s c) -> p s c", c=SLOTW)
            s0 = soff(p0) // SLOTW
            nc.sync.dma_start(
                out=out[:, p0:p0 + OUTB, :],
                in_=src[:, s0:s0 + OUTB, 1:WDIM + 1],
            )

        for rr in range(h - 2, max(h - 2 - XAHEAD, -1), -1):
            fetch_resid_bwd(rr)
        prev_combo = None
        for p in range(h - 2, -1, -1):
            if p - XAHEAD >= 0:
                fetch_resid_bwd(p - XAHEAD)
            # backward: resid = clean f[p] -> combo sums to g+1 -> scalar -1
            prev_combo = step(
                p,
                prev_combo if p < h - 2 else None,
                p + 1,
                -1.0,
            )
            if p % OUTB == 0:
                store_rows(p)
        # the top store batch includes row h-1 (g[h-1] = f[h-1], from forward)
    nc.compile()
    return nc


_NC_CACHE = {}


def _get_nc(h=H_FULL):
    if h not in _NC_CACHE:
        _NC_CACHE[h] = _build(h)
    return _NC_CACHE[h]


def _in_maps(x, W):
    n = x.shape[0]
    w1t = W[:, :, 1, :].transpose(1, 2, 0)  # [ci, k, co]
    wb = np.ascontiguousarray(
        np.broadcast_to(w1t, (2,) + w1t.shape).astype(np.float32)
    )
    return [
        {
            "xm1": np.ascontiguousarray((x[s] - 1.0).astype(np.float32)),
            "wb": wb,
        }
        for s in range(n)
    ]


def run(x, W, h=H_FULL, **kw):
    nc = _get_nc(h)
    res = run_bass_kernel_spmd(
        nc, _in_maps(x, W), core_ids=list(range(x.shape[0])), **kw
    )
    outs = np.stack([r["out"] for r in res.results], axis=0)
    return outs, res


def kernel(x, W, b):
    x = np.asarray(x)
    W = np.asarray(W)
    outs, _ = run(x, W, h=x.shape[2])
    return outs.astype(np.float32)



# revision 5
# speedup vs baseline: 1.3021x; 1.3021x over previous
"""Trainium2 Bass kernel for nn_DirectionalConvLayer.

Model (from the reference): per sample, a forward then backward scan over h;
each step = 3x3 conv on a single row (only the middle kernel row W[:,:,1,:]
contributes), + bias b, InstanceNorm over the row, ELU, + residual row. The
conv bias b cancels exactly under InstanceNorm and is never sent the device.

Sharding: data-parallel over batch n=8 -> one sample per NeuronCore, no
collectives. Each core runs the sequential 2*(h-1)-step scan on its sample.

Step math, with z = conv(prev row), m/v = row stats of z, rs = rsqrt(v+eps),
u = (z-m)*rs (standardized, so exp never overflows):
    f_dst = elu(u) + resid = [min(exp(u),1) - 1] + [relu(u)] + resid
          = em - 1 + ru*rs + resid
      em = exp(rs * min(z-m, 0)),  ru = max(z-m, 0)

Key structure (v4): the conv is linear, so the next step's conv input is a
128-partition "combo" slot holding q = ru*rs + (resid - 1) on partitions
0:64 and em on partitions 64:128; one K=128 matmul per tap computes
conv(q) + conv(em) in one shot. The combo sums to f (forward, pads 0) or
g+1 (backward, pads 1 on the q half); the per-channel constant offset is
absorbed exactly by InstanceNorm's mean subtraction.

Critical-path engineering (all ranks from the TimelineSim cost model):
- z is written TWICE by the PE (two PSUM banks, 6 cheap f16 matmuls): z1
  feeds DVE bn_stats, z2 feeds the ACT copy zc. With a single z, walrus
  merges the second reader's waits into a chained wait on the first
  reader's completion, serializing zc behind bn_stats (~400ns of path).
- zc writes an f16 copy of z; rd/ru/q/qm are then all-f16 TensorScalarPtr
  ops that hit the DVE 4x perf mode (127ns instead of 327).
- The f-stack is f16 (feeds backward residuals + seed convs); output rows
  are staged fp32 in a small ring written by Pool off the critical path.
- ACT per step: zc, Ln, Exp(rs) (scalar ops ~free), Exp(em). em writes the
  combo em half at partitions 64:128 directly (output base is free), so no
  em duplicate is needed: the Pool stack write reads em (combo, 64:128) +
  qm (64:128) at equal partition bases.

All ACT functions used (Ln, Exp, Copy) live in one activation-table set
(natural_log_exp_and_others); the default chooser would reload tables twice
per step (~2.6us/step), so _Bacc pins the choice.
"""

from contextlib import ExitStack

import numpy as np

import concourse.bacc as bacc
import concourse.bass as bass
import concourse.mybir as mybir
import concourse.tile as tile
from concourse.bass_utils import run_bass_kernel_spmd

F32 = mybir.dt.float32
F16 = mybir.dt.float16
AF = mybir.ActivationFunctionType
OP = mybir.AluOpType

EPS = 1e-5
C = 64          # channels
WDIM = 256      # row width
H_FULL = 256    # rows
SLOTW = WDIM + 2  # padded row slot width
XAHEAD = 4      # resid DMA prefetch distance
OUTB = 4        # out rows per store DMA
NCOMBO = 3      # combo ring depth
SBUFS = 3       # stats pool depth
EBUFS = 3       # elementwise pool depth
ZBUFS = 2       # z PSUM pool depth


class _Bacc(bacc.Bacc):
    """Bacc whose activation-table chooser is forced to the single set
    containing Ln and Exp (natural_log_exp_and_others); the default
    first-match rule alternates natural_log / exp_and_others, reloading
    ACT tables twice per scan step."""

    def insert_act_table_loads(self):
        import bass_rust as _bass_rust
        from concourse.hw_specs import get_activation_tables

        has_activation = any(
            isinstance(i, mybir.InstActivation)
            for b in self.main_func.blocks
            for i in b.instructions
        )
        if not has_activation:
            return
        want = {AF.Ln, AF.Exp, AF.Copy}
        tables = [
            (name, funcs if name == "natural_log_exp_and_others"
             else funcs - want)
            for name, funcs in get_activation_tables(self.m.arch).items()
        ]
        _bass_rust.insert_act_table_loads(self, tables)


def _build(h=H_FULL):
    half_rows = h // 2

    def hb(row):  # partition base of the stack half that owns `row`
        return 0 if row < half_rows else 64

    def soff(row):  # column offset of `row`'s slot within its half
        return (row % half_rows) * SLOTW

    nc = _Bacc("TRN2", target_bir_lowering=False, debug=False, num_devices=8)
    # xm1 = x - 1 (host-side); the -1 belongs inside q = ru*rs + resid - 1
    xm1 = nc.dram_tensor("xm1", [C, h, WDIM], F32, kind="ExternalInput").ap()
    # wb[half, ci, k, co] = W[co, ci, 1, k]; duplicated across both halves ->
    # combo matmuls contract q (parts 0:64) and em (parts 64:128) in one shot
    wb = nc.dram_tensor("wb", [2, C, 3, C], F32, kind="ExternalInput").ap()
    out = nc.dram_tensor("out", [C, h, WDIM], F32, kind="ExternalOutput").ap()

    with tile.TileContext(nc) as tc, ExitStack() as ctx:
        singles = ctx.enter_context(tc.tile_pool(name="singles", bufs=1))
        spool = ctx.enter_context(tc.tile_pool(name="stats", bufs=SBUFS))
        epool = ctx.enter_context(tc.tile_pool(name="elems", bufs=EBUFS))
        xpool = ctx.enter_context(tc.tile_pool(name="xrows", bufs=XAHEAD + 2))
        opool = ctx.enter_context(tc.tile_pool(name="oring", bufs=2))
        zpool = ctx.enter_context(tc.tile_pool(name="zpsum", bufs=ZBUFS, space="PSUM"))

        # f16 f-stack: forward rows f[0..h-1], consumed by backward residuals
        # and the seed convs. Output rows are staged fp32 in the oring pool.
        stack = singles.tile([128, half_rows * SLOTW], F16)
        w_stage = singles.tile([128, 3 * C], F32)
        w16 = singles.tile([128, 3 * C], F16)
        eps_t = singles.tile([128, 1], F32)
        nc.vector.memset(eps_t, EPS)

        # stage weights through a DVE copy so matmul weight deps are DVE ticks
        nc.sync.dma_start(out=w_stage, in_=wb)
        nc.vector.tensor_copy(w16, w_stage)

        # combo ring: persistent tiles. The q+em SUM over the two halves must
        # equal f (fwd) resp. g+1 (bwd) per column including pads, so the
        # phase pad value lives on the q half only; em-half pads stay 0.
        combos = [singles.tile([128, SLOTW], F16, name=f"combo{j}")
                  for j in range(NCOMBO)]
        for cb in combos:
            nc.vector.memset(cb[64:128, 0:1], 0.0)
            nc.vector.memset(cb[64:128, SLOTW - 1:SLOTW], 0.0)

        def set_pads(val):
            for cb in combos:
                nc.vector.memset(cb[0:64, 0:1], val)
                nc.vector.memset(cb[0:64, SLOTW - 1:SLOTW], val)

        # forward: q = ru*rs + (x-1) carries elu's -1, so the combo sums to
        # exactly f -> pads stay 0 and the stack write adds 0
        set_pads(0.0)

        # stack pad columns are zero (seed rows are conv'd directly)
        stack3 = stack.rearrange("p (s c) -> p s c", c=SLOTW)
        nc.vector.memset(stack3[:, :, 0:1], 0.0)
        nc.vector.memset(stack3[:, :, SLOTW - 1:SLOTW], 0.0)

        # f[0] = x[0] = xm1[0] + 1, staged then fixed up on DVE
        x0 = xpool.tile([128, WDIM], F32, tag="xr")
        nc.sync.dma_start(out=x0[0:64, :], in_=xm1[:, 0, :])
        nc.vector.tensor_scalar_add(stack[0:64, 1:WDIM + 1], x0[0:64, :], 1.0)

        resid_tiles = {}

        def fetch_xrow(row):
            xr = xpool.tile([128, WDIM], F32, tag="xr", name=f"xr{row}")
            nc.sync.dma_start(out=xr[0:64, :], in_=xm1[:, row, :])
            resid_tiles[row] = xr

        def fetch_resid_bwd(row):
            # backward resid is stack row `row` (clean f, f16); rows in the
            # upper half live at partitions 64:128 but the q-stt needs base-0
            # inputs -> stage through an SBUF->SBUF DMA. Lower half reads
            # the stack directly.
            if hb(row) == 0:
                resid_tiles[row] = None  # direct
                return
            xr = xpool.tile([128, WDIM], F16, tag="br", name=f"br{row}")
            nc.sync.dma_start(
                out=xr[0:64, :],
                in_=stack[64:128, soff(row) + 1:soff(row) + 1 + WDIM],
            )
            resid_tiles[row] = xr

        def resid_ap(row):
            t = resid_tiles.pop(row)
            if t is None:
                return stack[0:64, soff(row) + 1:soff(row) + 1 + WDIM]
            return t[0:64, :]

        ring = {"tile": None}

        def step(dst, src_combo, src_stack_row, backward):
            """One scan step. Conv input: either a combo ring slot (K=128,
            q+em) or a stack row (seed steps, K=64). Writes q/em into
            combo[dst % NCOMBO]; forward also writes f16 f into stack slot
            dst, backward writes fp32 g into the out ring."""
            hd = hb(dst)
            do = soff(dst)
            # z twice: z1 (bn_stats) then z2 (zc). Separate PSUM tiles give
            # both readers direct PE waits (no walrus reader-chaining).
            z1t = zpool.tile([128, WDIM], F32, tag="z", name=f"z{dst}")
            z2t = zpool.tile([128, WDIM], F32, tag="zb", name=f"zb{dst}")
            for zt in (z1t, z2t):
                zdst = zt[0:64, :]
                if src_combo is not None:
                    for k in range(3):
                        nc.tensor.matmul(
                            zdst,
                            lhsT=w16[:, k * C:(k + 1) * C],
                            rhs=src_combo[:, k:k + WDIM],
                            start=(k == 0),
                            stop=(k == 2),
                        )
                else:
                    hs = hb(src_stack_row)
                    so = soff(src_stack_row)
                    for k in range(3):
                        nc.tensor.matmul(
                            zdst,
                            lhsT=w16[hs:hs + 64, k * C:(k + 1) * C],
                            rhs=stack[hs:hs + 64, so + k:so + k + WDIM],
                            start=(k == 0),
                            stop=(k == 2),
                        )
            z1 = z1t[0:64, :]
            z2 = z2t[0:64, :]

            st6 = spool.tile([128, 6], F32, tag="st6", name=f"st{dst}")
            nc.vector.bn_stats(st6[0:64, :], z1)
            # f16 z copy on ACT, in parallel with bn_stats on DVE
            zc = epool.tile([128, WDIM], F16, tag="zc", name=f"zc{dst}")
            nc.scalar.activation(zc[0:64, :], z2, AF.Copy)
            mv = spool.tile([128, 2], F32, tag="mv", name=f"mv{dst}")
            nc.vector.bn_aggr(mv[0:64, :], st6[0:64, :])
            mean = mv[0:64, 0:1]
            var = mv[0:64, 1:2]
            # rd/ru before Ln in emission order so their zc wait encodes as
            # ACT>=zc (not chained through the later Ln/Exp ACT ops)
            rd = epool.tile([128, WDIM], F16, tag="rd", name=f"rd{dst}")
            nc.vector.tensor_scalar(
                rd[0:64, :], zc[0:64, :], mean, 0.0, OP.subtract, OP.min
            )
            ru = epool.tile([128, WDIM], F16, tag="ru", name=f"ru{dst}")
            nc.vector.tensor_scalar(
                ru[0:64, :], zc[0:64, :], mean, 0.0, OP.subtract, OP.max
            )
            lv = spool.tile([128, 1], F32, tag="lv", name=f"lv{dst}")
            nc.scalar.activation(lv[0:64, :], var, AF.Ln, bias=eps_t[0:64, :])
            rs = spool.tile([128, 1], F32, tag="rs", name=f"rs{dst}")
            nc.scalar.activation(rs[0:64, :], lv[0:64, :], AF.Exp, scale=-0.5)
            cb = combos[dst % NCOMBO]
            # em = exp(rs*min(z-m,0)) = min(exp(u),1), into combo em half
            # (partitions 64:128; ACT output base is free)
            nc.scalar.activation(
                cb[64:128, 1:WDIM + 1], rd[0:64, :], AF.Exp, scale=rs[0:64, :]
            )
            # q = ru*rs + resid(-1) into combo q half (partitions 0:64)
            resid = resid_ap(dst)
            nc.vector.scalar_tensor_tensor(
                cb[0:64, 1:WDIM + 1], ru[0:64, :], rs[0:64, :],
                resid, OP.mult, OP.add,
            )
            # qm: duplicate of q at partitions 64:128 so the Pool stack/ring
            # write reads same-base inputs (em is at 64:128 in the combo)
            qm = epool.tile([128, WDIM], F16, tag="qm", name=f"qm{dst}")
            nc.vector.scalar_tensor_tensor(
                qm[64:128, :], ru[0:64, :], rs[0:64, :],
                resid, OP.mult, OP.add,
            )
            em16 = cb[64:128, 1:WDIM + 1]
            if not backward:
                # stack <- f = em + q (f16), off the critical path on Pool
                nc.gpsimd.tensor_tensor(
                    stack[hd:hd + 64, do + 1:do + 1 + WDIM],
                    em16, qm[64:128, :], OP.add,
                )
                if dst == h - 1:
                    # f[h-1] is also output row h-1 (g[h-1] = f[h-1])
                    rt = opool.tile([128, OUTB * WDIM], F32, tag="ring",
                                    name=f"ring{dst // OUTB}")
                    ring["tile"] = rt
                    nc.gpsimd.tensor_tensor(
                        rt[64:128, (dst % OUTB) * WDIM:(dst % OUTB + 1) * WDIM],
                        em16, qm[64:128, :], OP.add,
                    )
            else:
                # out ring <- g = em + q - 1 (fp32)
                if dst % OUTB == OUTB - 1:
                    ring["tile"] = opool.tile([128, OUTB * WDIM], F32,
                                              tag="ring", name=f"ring{dst // OUTB}")
                rt = ring["tile"]
                sl = rt[64:128, (dst % OUTB) * WDIM:(dst % OUTB + 1) * WDIM]
                nc.gpsimd.tensor_tensor(sl, em16, qm[64:128, :], OP.add)
                nc.gpsimd.tensor_scalar_add(sl, sl, -1.0)
            return cb

        # ---- forward scan: f[i] = elu-step(f[i-1]) + x[i] ----
        for rr in range(1, min(1 + XAHEAD, h)):
            fetch_xrow(rr)
        prev_combo = None
        for i in range(1, h):
            if i + XAHEAD <= h - 1:
                fetch_xrow(i + XAHEAD)
            prev_combo = step(
                i,
                prev_combo if i > 1 else None,
                i - 1,
                backward=False,
            )

        # ---- backward scan: g[p] = elu-step(g[p+1]) + f[p] ----
        set_pads(1.0)  # backward: combo = g + 1 (q = ru*rs + f)

        def store_rows(p0):
            rt = ring["tile"]
            src = rt[64:128, :].rearrange("p (s c) -> p s c", c=WDIM)
            nc.sync.dma_start(
                out=out[:, p0:p0 + OUTB, :],
                in_=src[:, 0:OUTB, :],
            )

        for rr in range(h - 2, max(h - 2 - XAHEAD, -1), -1):
            fetch_resid_bwd(rr)
        prev_combo = None
        for p in range(h - 2, -1, -1):
            if p - XAHEAD >= 0:
                fetch_resid_bwd(p - XAHEAD)
            prev_combo = step(
                p,
                prev_combo if p < h - 2 else None,
                p + 1,
                backward=True,
            )
            if p % OUTB == 0:
                store_rows(p)
        # the top store batch includes row h-1 (g[h-1] = f[h-1], from forward)
    nc.compile()
    return nc


_NC_CACHE = {}


def _get_nc(h=H_FULL):
    if h not in _NC_CACHE:
        _NC_CACHE[h] = _build(h)
    return _NC_CACHE[h]


def _in_maps(x, W):
    n = x.shape[0]
    w1t = W[:, :, 1, :].transpose(1, 2, 0)  # [ci, k, co]
    wb = np.ascontiguousarray(
        np.broadcast_to(w1t, (2,) + w1t.shape).astype(np.float32)
    )
    return [
        {
            "xm1": np.ascontiguousarray((x[s] - 1.0).astype(np.float32)),
            "wb": wb,
        }
        for s in range(n)
    ]


def run(x, W, h=H_FULL, **kw):
    nc = _get_nc(h)
    res = run_bass_kernel_spmd(
        nc, _in_maps(x, W), core_ids=list(range(x.shape[0])), **kw
    )
    outs = np.stack([r["out"] for r in res.results], axis=0)
    return outs, res


def kernel(x, W, b):
    x = np.asarray(x)
    W = np.asarray(W)
    outs, _ = run(x, W, h=x.shape[2])
    return outs.astype(np.float32)


# revision 6
# speedup vs baseline: 1.3026x; 1.0003x over previous
"""Trainium2 Bass kernel for nn_DirectionalConvLayer.

Model (from the reference): per sample, a forward then backward scan over h;
each step = 3x3 conv on a single row (only the middle kernel row W[:,:,1,:]
contributes), + bias b, InstanceNorm over the row, ELU, + residual row. The
conv bias b cancels exactly under InstanceNorm and is never sent the device.

Sharding: data-parallel over batch n=8 -> one sample per NeuronCore, no
collectives. Each core runs the sequential 2*(h-1)-step scan on its sample.

Step math, with z = conv(prev row), m/v = row stats of z, rs = rsqrt(v+eps),
u = (z-m)*rs (standardized, so exp never overflows):
    f_dst = elu(u) + resid = [min(exp(u),1) - 1] + [relu(u)] + resid
          = em - 1 + ru*rs + resid
      em = exp(rs * min(z-m, 0)),  ru = max(z-m, 0)

Key structure (v5): the conv is linear, so the scan state row f = (elu+1) +
(resid-1) is split: the nonlinear part (elu+1 = q' + em, q' = ru*rs on
partitions 0:64, em on 64:128 of a 128-partition "combo" slot) is produced
on the critical path, while conv(resid-part) is PRE-ACCUMULATED into the
step's PSUM tiles by the PE during the previous step (resid rows are known
ahead: x rows forward, f-stack rows backward). The dependent matmuls then
accumulate conv(combo) on top. Per-channel constant offsets (the -1s, and
conv(f) vs conv(f-1)) are uniform across each padded row and are absorbed
exactly by InstanceNorm's mean subtraction, so all pads stay 0 in both
phases.

Critical-path engineering (all from the TimelineSim cost model):
- z is written TWICE by the PE (separate PSUM banks): z1 feeds DVE
  bn_stats, z2 feeds the ACT f16 copy zc. With a single z, walrus merges
  the second reader's waits into a chained wait on the first reader's
  completion, serializing them (~400ns of path).
- rd/ru/q' are single-tensor-input TensorScalar ops on all-f16 SBUF
  operands -> DVE 4x perf mode (127ns each). The two-tensor-input stt
  form has no fast modes (327ns), so it is used only for qm (the Pool
  stack/ring feed), which is off the critical path.
- The f-stack is f16 (backward residuals + seed/precompute convs read it);
  output rows are staged fp32 in a small ring written by Pool.
- ACT per step: zc, Ln, Exp(rs) (scalar ops ~free), Exp(em into the combo
  em half at partitions 64:128; output base is free).

All ACT functions used (Ln, Exp, Copy) live in one activation-table set
(natural_log_exp_and_others); the default chooser would reload tables twice
per step (~2.6us/step), so _Bacc pins the choice.
"""

from contextlib import ExitStack

import numpy as np

import concourse.bacc as bacc
import concourse.bass as bass
import concourse.mybir as mybir
import concourse.tile as tile
from concourse.bass_utils import run_bass_kernel_spmd

F32 = mybir.dt.float32
F16 = mybir.dt.float16
AF = mybir.ActivationFunctionType
OP = mybir.AluOpType

EPS = 1e-5
C = 64          # channels
WDIM = 256      # row width
H_FULL = 256    # rows
SLOTW = WDIM + 2  # padded row slot width
XAHEAD = 4      # resid DMA prefetch distance
NXSLOT = XAHEAD + 2  # persistent padded x-row slots
OUTB = 4        # out rows per store DMA
NCOMBO = 3      # combo ring depth
SBUFS = 3       # stats pool depth
EBUFS = 3       # elementwise pool depth
ZBUFS = 2       # z PSUM pool depth


class _Bacc(bacc.Bacc):
    """Bacc whose activation-table chooser is forced to the single set
    containing Ln and Exp (natural_log_exp_and_others); the default
    first-match rule alternates natural_log / exp_and_others, reloading
    ACT tables twice per scan step."""

    def insert_act_table_loads(self):
        import bass_rust as _bass_rust
        from concourse.hw_specs import get_activation_tables

        has_activation = any(
            isinstance(i, mybir.InstActivation)
            for b in self.main_func.blocks
            for i in b.instructions
        )
        if not has_activation:
            return
        want = {AF.Ln, AF.Exp, AF.Copy}
        tables = [
            (name, funcs if name == "natural_log_exp_and_others"
             else funcs - want)
            for name, funcs in get_activation_tables(self.m.arch).items()
        ]
        _bass_rust.insert_act_table_loads(self, tables)


def _build(h=H_FULL):
    half_rows = h // 2

    def hb(row):  # partition base of the stack half that owns `row`
        return 0 if row < half_rows else 64

    def soff(row):  # column offset of `row`'s slot within its half
        return (row % half_rows) * SLOTW

    nc = _Bacc("TRN2", target_bir_lowering=False, debug=False, num_devices=8)
    # xm1 = x - 1 in f16 (host-side); the -1 belongs to the resid-part of f
    xm1 = nc.dram_tensor("xm1", [C, h, WDIM], F16, kind="ExternalInput").ap()
    # wb[half, ci, k, co] = W[co, ci, 1, k]; duplicated across both halves ->
    # combo matmuls contract q' (parts 0:64) and em (parts 64:128) in one shot
    wb = nc.dram_tensor("wb", [2, C, 3, C], F32, kind="ExternalInput").ap()
    out = nc.dram_tensor("out", [C, h, WDIM], F32, kind="ExternalOutput").ap()

    with tile.TileContext(nc) as tc, ExitStack() as ctx:
        singles = ctx.enter_context(tc.tile_pool(name="singles", bufs=1))
        spool = ctx.enter_context(tc.tile_pool(name="stats", bufs=SBUFS))
        epool = ctx.enter_context(tc.tile_pool(name="elems", bufs=EBUFS))
        xpool = ctx.enter_context(tc.tile_pool(name="xrows", bufs=XAHEAD + 2))
        opool = ctx.enter_context(tc.tile_pool(name="oring", bufs=2))
        zpool = ctx.enter_context(tc.tile_pool(name="zpsum", bufs=ZBUFS, space="PSUM"))

        # f16 f-stack: forward rows f[0..h-1], consumed by backward residuals
        # and the seed/precompute convs. Output rows go through oring (fp32).
        stack = singles.tile([128, half_rows * SLOTW], F16)
        w_stage = singles.tile([128, 3 * C], F32)
        w16 = singles.tile([128, 3 * C], F16)
        eps_t = singles.tile([128, 1], F32)
        nc.vector.memset(eps_t, EPS)

        # stage weights through a DVE copy so matmul weight deps are DVE ticks
        nc.sync.dma_start(out=w_stage, in_=wb)
        nc.vector.tensor_copy(w16, w_stage)

        # combo ring: persistent tiles; pads stay 0 in both phases (the conv
        # input row f resp. g+const is padded with 0 across combo + resid).
        combos = [singles.tile([128, SLOTW], F16, name=f"combo{j}")
                  for j in range(NCOMBO)]
        for cb in combos:
            nc.vector.memset(cb[:, 0:1], 0.0)
            nc.vector.memset(cb[:, SLOTW - 1:SLOTW], 0.0)

        # stack pad columns are zero
        stack3 = stack.rearrange("p (s c) -> p s c", c=SLOTW)
        nc.vector.memset(stack3[:, :, 0:1], 0.0)
        nc.vector.memset(stack3[:, :, SLOTW - 1:SLOTW], 0.0)

        # persistent padded x-row slots (f16), pads zeroed once; ring of
        # NXSLOT entries indexed by row % NXSLOT
        xslots = [singles.tile([128, SLOTW], F16, name=f"xs{j}")
                  for j in range(NXSLOT)]
        for xs in xslots:
            nc.vector.memset(xs[0:64, 0:1], 0.0)
            nc.vector.memset(xs[0:64, SLOTW - 1:SLOTW], 0.0)

        def fetch_xrow(row):
            xs = xslots[row % NXSLOT]
            nc.sync.dma_start(out=xs[0:64, 1:WDIM + 1], in_=xm1[:, row, :])

        # f[0] = x[0] = xm1[0] + 1 into stack row 0
        fetch_xrow(0)
        nc.vector.tensor_scalar_add(
            stack[0:64, 1:WDIM + 1], xslots[0][0:64, 1:WDIM + 1], 1.0
        )

        resid_tiles = {}

        def fetch_resid_bwd(row):
            # backward qm-resid is stack row `row` (clean f, f16); rows in
            # the upper half live at partitions 64:128 but the qm-stt needs
            # base-0 inputs -> stage through an SBUF->SBUF DMA. Lower half
            # reads the stack directly.
            if hb(row) == 0:
                resid_tiles[row] = None  # direct
                return
            xr = xpool.tile([128, WDIM], F16, tag="br", name=f"br{row}")
            nc.sync.dma_start(
                out=xr[0:64, :],
                in_=stack[64:128, soff(row) + 1:soff(row) + 1 + WDIM],
            )
            resid_tiles[row] = xr

        def resid_ap_bwd(row):
            t = resid_tiles.pop(row)
            if t is None:
                return stack[0:64, soff(row) + 1:soff(row) + 1 + WDIM]
            return t[0:64, :]

        zmap = {}

        def pre_conv(key, rhs_base, rhs_off, full):
            """Allocate PSUM z1/z2 for step `key` and accumulate the
            precomputable conv(resid-part): 3 taps from a padded K=64 f16
            row at partitions rhs_base with column offset rhs_off. If
            `full`, this is a seed step: the precompute IS the whole conv
            (stop on the last tap)."""
            z1t = zpool.tile([128, WDIM], F32, tag="z", name=f"z{key}")
            z2t = zpool.tile([128, WDIM], F32, tag="zb", name=f"zb{key}")
            zmap[key] = (z1t, z2t)
            for zt, src in ((z1t, rhs_base), (z2t, rhs_base)):
                for k in range(3):
                    nc.tensor.matmul(
                        zt[0:64, :],
                        lhsT=w16[src[0]:src[0] + 64, k * C:(k + 1) * C],
                        rhs=src[1][src[0]:src[0] + 64,
                                   rhs_off + k:rhs_off + k + WDIM],
                        start=(k == 0),
                        stop=(full and k == 2),
                    )

        def dep_conv(key, src_combo):
            """Accumulate conv(combo) (the elu+1 part) onto the
            pre-accumulated PSUM tiles of step `key`."""
            z1t, z2t = zmap[key]
            for zt in (z1t, z2t):
                for k in range(3):
                    nc.tensor.matmul(
                        zt[0:64, :],
                        lhsT=w16[:, k * C:(k + 1) * C],
                        rhs=src_combo[:, k:k + WDIM],
                        start=False,
                        stop=(k == 2),
                    )

        ring = {"tile": None}

        def step(dst, resid, backward):
            """Post-conv work for step dst (z1/z2 are complete in PSUM).
            Writes q'/em into combo[dst % NCOMBO]; forward also writes the
            f16 f row into the stack, backward writes fp32 g into the out
            ring. `resid` is the base-0 f16 AP of the resid row (x row
            forward, f row backward) for the qm (stack/ring feed) only."""
            hd = hb(dst)
            do = soff(dst)
            z1t, z2t = zmap.pop(dst)
            z1 = z1t[0:64, :]
            z2 = z2t[0:64, :]

            st6 = spool.tile([128, 6], F32, tag="st6", name=f"st{dst}")
            nc.vector.bn_stats(st6[0:64, :], z1)
            # f16 z copy on ACT, in parallel with bn_stats on DVE
            zc = epool.tile([128, WDIM], F16, tag="zc", name=f"zc{dst}")
            nc.scalar.activation(zc[0:64, :], z2, AF.Copy)
            mv = spool.tile([128, 2], F32, tag="mv", name=f"mv{dst}")
            nc.vector.bn_aggr(mv[0:64, :], st6[0:64, :])
            mean = mv[0:64, 0:1]
            var = mv[0:64, 1:2]
            # rd/ru before Ln in emission order so their zc wait encodes as
            # ACT>=zc (not chained through the later Ln/Exp ACT ops)
            rd = epool.tile([128, WDIM], F16, tag="rd", name=f"rd{dst}")
            nc.vector.tensor_scalar(
                rd[0:64, :], zc[0:64, :], mean, 0.0, OP.subtract, OP.min
            )
            ru = epool.tile([128, WDIM], F16, tag="ru", name=f"ru{dst}")
            nc.vector.tensor_scalar(
                ru[0:64, :], zc[0:64, :], mean, 0.0, OP.subtract, OP.max
            )
            lv = spool.tile([128, 1], F32, tag="lv", name=f"lv{dst}")
            nc.scalar.activation(lv[0:64, :], var, AF.Ln, bias=eps_t[0:64, :])
            rs = spool.tile([128, 1], F32, tag="rs", name=f"rs{dst}")
            nc.scalar.activation(rs[0:64, :], lv[0:64, :], AF.Exp, scale=-0.5)
            cb = combos[dst % NCOMBO]
            # em = exp(rs*min(z-m,0)) = min(exp(u),1), into combo em half
            # (partitions 64:128; ACT output base is free)
            nc.scalar.activation(
                cb[64:128, 1:WDIM + 1], rd[0:64, :], AF.Exp, scale=rs[0:64, :]
            )
            # q' = ru*rs into combo q half (partitions 0:64): single-src
            # TensorScalar -> DVE 4x mode
            nc.vector.tensor_scalar(
                cb[0:64, 1:WDIM + 1], ru[0:64, :], rs[0:64, 0:1], 0.0,
                OP.mult, OP.add,
            )
            # qm = ru*rs + resid at partitions 64:128 (same base as em) for
            # the 2-input Pool stack/ring write; off the critical path
            qm = epool.tile([128, WDIM], F16, tag="qm", name=f"qm{dst}")
            nc.vector.scalar_tensor_tensor(
                qm[64:128, :], ru[0:64, :], rs[0:64, 0:1],
                resid, OP.mult, OP.add,
            )
            em16 = cb[64:128, 1:WDIM + 1]
            if not backward:
                # stack <- f = em + qm (f16): em+ru*rs+x-1 = elu+x
                nc.gpsimd.tensor_tensor(
                    stack[hd:hd + 64, do + 1:do + 1 + WDIM],
                    em16, qm[64:128, :], OP.add,
                )
                if dst == h - 1:
                    # f[h-1] is also output row h-1 (g[h-1] = f[h-1])
                    rt = opool.tile([128, OUTB * WDIM], F32, tag="ring",
                                    name=f"ring{dst // OUTB}")
                    ring["tile"] = rt
                    nc.gpsimd.tensor_tensor(
                        rt[64:128, (dst % OUTB) * WDIM:(dst % OUTB + 1) * WDIM],
                        em16, qm[64:128, :], OP.add,
                    )
            else:
                # out ring <- g = em + qm - 1 (fp32): qm = ru*rs + f
                if dst % OUTB == OUTB - 1:
                    ring["tile"] = opool.tile([128, OUTB * WDIM], F32,
                                              tag="ring", name=f"ring{dst // OUTB}")
                rt = ring["tile"]
                sl = rt[64:128, (dst % OUTB) * WDIM:(dst % OUTB + 1) * WDIM]
                nc.gpsimd.tensor_tensor(sl, em16, qm[64:128, :], OP.add)
                nc.gpsimd.tensor_scalar_add(sl, sl, -1.0)
            return cb

        # ---- forward scan: f[i] = elu-step(f[i-1]) + x[i] ----
        for rr in range(1, min(1 + XAHEAD, h)):
            fetch_xrow(rr)
        # seed: step 1's conv is entirely precomputable (f[0] = stack row 0)
        pre_conv(1, (hb(0), stack), soff(0), full=True)
        prev_combo = None
        for i in range(1, h):
            if i + XAHEAD <= h - 1:
                fetch_xrow(i + XAHEAD)
            if prev_combo is not None:
                dep_conv(i, prev_combo)
            prev_combo = step(i, xslots[i % NXSLOT][0:64, 1:WDIM + 1],
                              backward=False)
            if i + 1 < h:
                # precompute conv(xm1 row i) for step i+1 (f[i] resid part)
                pre_conv(i + 1, (0, xslots[i % NXSLOT]), 0, full=False)

        # ---- backward scan: g[p] = elu-step(g[p+1]) + f[p] ----
        for rr in range(h - 2, max(h - 2 - XAHEAD, -1), -1):
            fetch_resid_bwd(rr)
        # seed: step h-2's conv is entirely precomputable (g[h-1] = f[h-1])
        pre_conv(h - 2, (hb(h - 1), stack), soff(h - 1), full=True)
        prev_combo = None

        def store_rows(p0):
            rt = ring["tile"]
            src = rt[64:128, :].rearrange("p (s c) -> p s c", c=WDIM)
            nc.sync.dma_start(
                out=out[:, p0:p0 + OUTB, :],
                in_=src[:, 0:OUTB, :],
            )

        for p in range(h - 2, -1, -1):
            if p - XAHEAD >= 0:
                fetch_resid_bwd(p - XAHEAD)
            if prev_combo is not None:
                dep_conv(p, prev_combo)
            prev_combo = step(p, resid_ap_bwd(p), backward=True)
            if p - 1 >= 0:
                # precompute conv(stack row p) for step p-1 (g[p] resid part)
                pre_conv(p - 1, (hb(p), stack), soff(p), full=False)
            if p % OUTB == 0:
                store_rows(p)
        # the top store batch includes row h-1 (g[h-1] = f[h-1], from forward)
    nc.compile()
    return nc


_NC_CACHE = {}


def _get_nc(h=H_FULL):
    if h not in _NC_CACHE:
        _NC_CACHE[h] = _build(h)
    return _NC_CACHE[h]


def _in_maps(x, W):
    n = x.shape[0]
    w1t = W[:, :, 1, :].transpose(1, 2, 0)  # [ci, k, co]
    wb = np.ascontiguousarray(
        np.broadcast_to(w1t, (2,) + w1t.shape).astype(np.float32)
    )
    xm1 = (x - 1.0).astype(np.float16)
    return [
        {"xm1": np.ascontiguousarray(xm1[s]), "wb": wb}
        for s in range(n)
    ]


def run(x, W, h=H_FULL, **kw):
    nc = _get_nc(h)
    res = run_bass_kernel_spmd(
        nc, _in_maps(x, W), core_ids=list(range(x.shape[0])), **kw
    )
    outs = np.stack([r["out"] for r in res.results], axis=0)
    return outs, res


def kernel(x, W, b):
    x = np.asarray(x)
    W = np.asarray(W)
    outs, _ = run(x, W, h=x.shape[2])
    return outs.astype(np.float32)
